# revision 25
# baseline (speedup 1.0000x reference)
"""Multi-head attention (N=4, S=T=2048, E=1024, H=16) on 8 trn2 NeuronCores.

Sharding: 8 cores = 4 batches x 2 head-groups (Megatron-style over heads).
Each core computes 8 heads of one batch and a partial output projection;
the host sums the two partials per batch and adds the output bias.

v3 schedule: the softmax exp stream on the Scalar engine (~283us) and
the matmul stream on the Tensor engine (~330us) are near co-bottlenecks,
so the kernel runs as one software-pipelined loop over 16 windows
(s-chunk x head-pair), each 16 t-block iterations:

  - scores+exp for window w are emitted in window w's iterations;
  - A.V for window w is deferred into window w+1's iterations (e-tiles
    stashed in SBUF), so the PE never drains the ACT backlog at window
    boundaries;
  - V/Q projections and the out-projection fill the remaining PE slack
    (V in the prologue + first two windows, Q chunk nn+1 during nn,
    out-projection of chunk nn-1 during nn+1).
"""
import os
import sys

for _p in ("/opt/trn_rl_repo", "/root/.axon_site/_ro/trn_rl_repo"):
    if os.path.isdir(_p) and _p not in sys.path:
        sys.path.append(_p)

import numpy as np
import ml_dtypes

import concourse.bass as bass
import concourse.mybir as mybir
import concourse.tile as tile
from concourse import bacc
from concourse.bass_utils import run_bass_kernel_spmd

F32 = mybir.dt.float32
BF16 = mybir.dt.bfloat16
AF = mybir.ActivationFunctionType

E_FULL = 1024
H_FULL = 16
HD = 64
N_FULL, S_FULL, T_FULL = 4, 2048, 2048
N_CORES = 8

# Set by the test harness to collect a profile.
TRACE = False
TRACE_KW = {}
LAST_RESULT = [None]


def _build_nc(E, S, T, NH):
    """Per-core kernel: E model dim, S query rows, T kv rows, NH heads
    (head dim 64). DG = NH*64 must be a multiple of 128."""
    DG = NH * HD
    EB = E // 128          # contraction blocks for projections
    DB = DG // 128         # head-pair blocks (2 heads of 64 each)
    TB = T // 128          # kv 128-blocks
    NQC = S // 512         # query 512-chunks
    NKC = T // 512         # kv 512-chunks
    NHP = NH // 2          # head pairs
    VW = NH * (HD + 1)     # V-aug width (64 cols + ones row per head)
    XD = BF16
    assert DG % 128 == 0 and S % 512 == 0 and NHP == 4 and NQC == 4

    nc = bacc.Bacc(None)
    xqT = nc.dram_tensor("xqT", [E, S], XD, kind="ExternalInput")
    xkT = nc.dram_tensor("xkT", [E, T], XD, kind="ExternalInput")
    xvT = nc.dram_tensor("xvT", [E, T], XD, kind="ExternalInput")
    wqT = nc.dram_tensor("wqT", [E, DG], XD, kind="ExternalInput")
    wkT = nc.dram_tensor("wkT", [E, DG], XD, kind="ExternalInput")
    wvT = nc.dram_tensor("wvT", [E, DG], XD, kind="ExternalInput")
    wpT = nc.dram_tensor("wpT", [DG, E], XD, kind="ExternalInput")
    bq = nc.dram_tensor("bq", [DG], F32, kind="ExternalInput")
    bk = nc.dram_tensor("bk", [DG], F32, kind="ExternalInput")
    bv = nc.dram_tensor("bv", [DG], F32, kind="ExternalInput")
    out = nc.dram_tensor("out", [S, E], F32, kind="ExternalOutput")

    xqR = xqT.rearrange("(eb p) s -> p eb s", p=128)
    xkR = xkT.rearrange("(eb p) t -> p eb t", p=128)
    xvR = xvT.rearrange("(eb p) t -> p eb t", p=128)

    with tile.TileContext(nc) as tc:
        with (
            tc.tile_pool(name="const", bufs=1) as cpool,
            tc.tile_pool(name="persist", bufs=1) as ppool,
            tc.tile_pool(name="xsp", bufs=2) as xsp,
            tc.tile_pool(name="xvp", bufs=2) as xvp,
            tc.tile_pool(name="stp", bufs=2, space="PSUM") as stp,
            tc.tile_pool(name="ytp", bufs=2, space="PSUM") as ytp,
            tc.tile_pool(name="opp", bufs=2, space="PSUM") as opp,
            tc.tile_pool(name="epool", bufs=18) as epool,
            tc.tile_pool(name="ypool", bufs=2) as ypool,
            tc.tile_pool(name="npool", bufs=2) as npool,
            tc.tile_pool(name="n1pool", bufs=1) as n1pool,
            tc.tile_pool(name="opool", bufs=2) as opool,
        ):
            # ---- DMA: critical path first (wk, xk c0/c1), 2-way splits
            # so transfers spread over more DMA engines ----
            bq_sb = cpool.tile([128, DB], F32, tag="bq")
            bk_sb = cpool.tile([128, DB], F32, tag="bk")
            bv_sb = cpool.tile([HD, NH], F32, tag="bv")
            nc.sync.dma_start(bq_sb[:], bq.rearrange("(db p) -> p db", p=128))
            nc.sync.dma_start(bk_sb[:], bk.rearrange("(db p) -> p db", p=128))
            nc.sync.dma_start(bv_sb[:], bv.rearrange("(h p) -> p h", p=HD))

            wk_sb = cpool.tile([128, EB, DG], XD, tag="wk")
            wq_sb = cpool.tile([128, EB, DG], XD, tag="wq")
            wv_sb = cpool.tile([128, EB, DG], XD, tag="wv")
            wkR = wkT.rearrange("(eb p) d -> p eb d", p=128)
            wqR = wqT.rearrange("(eb p) d -> p eb d", p=128)
            wvR = wvT.rearrange("(eb p) d -> p eb d", p=128)
            # per-eb splits: the first K matmul only waits for wk[eb=0]
            for h in range(EB):
                nc.sync.dma_start(wk_sb[:, h : h + 1, :], wkR[:, h : h + 1, :])

            def dma_xchunk(dst_tile, src_r, pc, splits=2):
                step = EB // splits
                for h in range(splits):
                    nc.sync.dma_start(
                        dst_tile[:, step * h : step * (h + 1), :],
                        src_r[
                            :, step * h : step * (h + 1),
                            pc * 512 : (pc + 1) * 512,
                        ],
                    )

            # Warm the ACT exp table during the DMA lead-in.
            warm = n1pool.tile([1, 8], F32, tag="warm", name="warm")
            nc.vector.memset(warm[:], 0.0)
            nc.scalar.activation(warm[:], warm[:], AF.Exp, scale=1.0)
            # Warm the PE clock (HAM un-throttles after ~3.4us of
            # sustained matmul activity): dummy matmuls on a zeroed
            # tile while the input DMAs stream; the result is never read.
            wmm = n1pool.tile([128, 512], XD, tag="wmm", name="wmm")
            nc.vector.memset(wmm[:], 0.0)
            wps = opp.tile([128, 512], F32, tag="op", name="wps")
            for _ in range(28):
                nc.tensor.matmul(wps[:], wmm[:, 0:128], wmm[:], start=True, stop=True)

            qt_sb = ppool.tile([128, DB, S], XD, tag="qt")    # Q^T [d, s]
            kt_sb = ppool.tile([128, DB, T], XD, tag="kt")    # K^T [d, t]
            v_sb = ppool.tile([128, TB, VW], BF16, tag="v")   # V [t, d]+ones

            # ---- emitters ----
            def proj_group(x_t, w_sb, b_sb, o_sb, pc, db, kbr=None):
                """d-128-block psum group of a projection chunk; kbr
                restricts to a kb subrange (for spreading)."""
                kbr = kbr if kbr is not None else range(EB)
                if kbr.start == 0:
                    proj_group.ps[db] = opp.tile([128, 512], F32, tag="op", name="psq")
                ps = proj_group.ps[db]
                for kb in kbr:
                    nc.tensor.matmul(
                        ps[:],
                        w_sb[:, kb, db * 128 : (db + 1) * 128],
                        x_t[:, kb, :],
                        start=(kb == 0),
                        stop=(kb == EB - 1),
                    )
                if kbr.stop == EB:
                    nc.vector.tensor_scalar_add(
                        o_sb[:, db, pc * 512 : (pc + 1) * 512],
                        ps[:],
                        b_sb[:, db : db + 1],
                    )
            proj_group.ps = {}

            def v_block(xv_t, tb):
                """V[t-block, all heads] into v_sb + the ones row; xv_t is
                the staged [128, EB, 512] chunk containing this t-block."""
                ps = opp.tile([128, DG], F32, tag="op", name="psv")
                for kb in range(EB):
                    nc.tensor.matmul(
                        ps[:],
                        xv_t[:, kb, (tb % 4) * 128 : (tb % 4 + 1) * 128],
                        wv_sb[:, kb, :],
                        start=(kb == 0),
                        stop=(kb == EB - 1),
                    )
                nc.vector.tensor_copy(
                    v_sb[:, tb, :].rearrange("p (h w) -> p h w", w=HD + 1)[
                        :, :, 0:HD
                    ],
                    ps[:].rearrange("p (h w) -> p h w", w=HD),
                )
                nc.vector.memset(
                    v_sb[:, tb, :].rearrange("p (h w) -> p h w", w=HD + 1)[
                        :, :, HD : HD + 1
                    ],
                    1.0,
                )

            def normalize(ye, hpair, yt_sb):
                for hi, h in ((0, hpair[0]), (1, hpair[1])):
                    db_, rh = h // 2, (h % 2) * 64
                    sp = n1pool.tile([128, 4], F32, tag="sp", name="sp")
                    nc.sync.dma_start(sp[:], ye[hi][64:65, :])
                    nc.vector.reciprocal(sp[:], sp[:])
                    rs = n1pool.tile([1, 512], F32, tag="rs", name="rs")
                    nc.sync.dma_start(rs[:], sp[:])
                    rbc = n1pool.tile([64, 512], F32, tag="rbc", name="rbc")
                    nc.gpsimd.partition_broadcast(rbc[:], rs[:])
                    if rh == 0:
                        dst = yt_sb[0:64, db_, :]
                        nc.vector.tensor_tensor(
                            dst, ye[hi][0:64, :], rbc[:], mybir.AluOpType.mult
                        )
                        nc.vector.tensor_scalar_add(dst, dst, bv_sb[:, h : h + 1])
                    else:
                        tmp = n1pool.tile([64, 512], XD, tag="ytmp", name="tmp")
                        nc.vector.tensor_tensor(
                            tmp[:], ye[hi][0:64, :], rbc[:], mybir.AluOpType.mult
                        )
                        nc.vector.tensor_scalar_add(tmp[:], tmp[:], bv_sb[:, h : h + 1])
                        # partition shift 0-63 -> 64-127 (DMA only)
                        nc.sync.dma_start(yt_sb[64:128, db_, :], tmp[:])

            def outproj_group(yt_sb, s0, sb, jc):
                op = opp.tile([128, 512], F32, tag="op", name="op")
                for ib in range(DB):
                    nc.tensor.matmul(
                        op[:],
                        yt_sb[:, ib, sb * 128 : (sb + 1) * 128],
                        wp_sb[:, ib, jc * 512 : (jc + 1) * 512],
                        start=(ib == 0),
                        stop=(ib == DB - 1),
                    )
                ob = opool.tile([128, 512], F32, tag="ob")
                nc.vector.tensor_copy(ob[:], op[:])
                nc.sync.dma_start(
                    out[
                        s0 + sb * 128 : s0 + (sb + 1) * 128,
                        jc * 512 : (jc + 1) * 512,
                    ],
                    ob[:],
                )

            # ---- prologue: K projection, Q chunk 0, V blocks 0-3 ----
            # xs-pool rotation order (bufs=2) is chosen so each new
            # chunk's DMA only WAR-waits on already-emitted consumers:
            # xk0, xk1, [K0], xk2, [K1], xk3, [K2], xq0, [K3], [Q0].
            xk_t = []
            for pc in range(2):
                t = xsp.tile([128, EB, 512], XD, tag="xs", name=f"xk{pc}")
                dma_xchunk(t, xkR, pc, splits=4 if pc == 0 else 2)
                xk_t.append(t)
            for h in range(4):  # wq/wv after the first xk chunks
                nc.sync.dma_start(
                    wq_sb[:, 2 * h : 2 * h + 2, :], wqR[:, 2 * h : 2 * h + 2, :]
                )
            for h in range(4):
                nc.sync.dma_start(
                    wv_sb[:, 2 * h : 2 * h + 2, :], wvR[:, 2 * h : 2 * h + 2, :]
                )
            xv_t = xvp.tile([128, EB, 512], XD, tag="xv", name="xv0")
            dma_xchunk(xv_t, xvR, 0)

            xq_t = None
            for pc in range(NKC):
                for db in range(DB):
                    proj_group(xk_t[pc], wk_sb, bk_sb, kt_sb, pc, db)
                if pc + 2 < NKC:
                    t = xsp.tile([128, EB, 512], XD, tag="xs", name=f"xk{pc+2}")
                    dma_xchunk(t, xkR, pc + 2)
                    xk_t.append(t)
                elif pc + 2 == NKC:
                    xq_t = xsp.tile([128, EB, 512], XD, tag="xs", name="xq0")
                    dma_xchunk(xq_t, xqR, 0)
            # Only Q0's db0 group is needed before window 0 (scores of
            # window w read just qt[:, hp, :]); db1-3 defer into windows.
            proj_group(xq_t, wq_sb, bq_sb, qt_sb, 0, 0)
            for tb in range(4):
                v_block(xv_t, tb)
            xv_next = xvp.tile([128, EB, 512], XD, tag="xv", name="xv1")
            dma_xchunk(xv_next, xvR, 1)
            wp_sb = cpool.tile([128, DB, E], XD, tag="wp")
            nc.sync.dma_start(wp_sb[:], wpT.rearrange("(db p) e -> p db e", p=128))

            # ---- main pipeline: 16 windows x 16 iterations ----
            # prev holds window w-1's state while its A.V runs in w.
            windows = [(nn, hp) for nn in range(NQC) for hp in range(NHP)]
            prev = None          # (e_tiles, h0, h1, yt0, yt1, nn)
            norm_q = []          # pending (ye, hpair, yt_sb)
            yt_sb_of = {}        # nn -> SBUF Y tile
            xq_of = {0: xq_t}    # staged xq chunks
            # V blocks 4..15 spread over windows 0-1 (iteration -> tb),
            # placed so V(tb) lands before its A.V use in the next window.
            v_sched = {0: {0: 4, 2: 5, 5: 6, 7: 7, 10: 8, 12: 9, 15: 10},
                       1: {0: 11, 3: 12, 6: 13, 9: 14, 12: 15}}
            # out-projection of chunk nn-1: 8 groups over hp1/hp2/hp3.
            # For nn3 keep hp3 clear: the last window's inline A.V parks
            # its accumulators in opp's psum banks.
            op_sched = {1: {2: 0, 7: 1, 12: 2}, 2: {2: 3, 7: 4, 12: 5},
                        3: {4: 6, 10: 7}}
            op_sched_last = {1: {2: 0, 6: 1, 10: 2, 14: 3},
                             2: {2: 4, 6: 5, 10: 6, 14: 7}}
            # Q-projection groups: one (chunk, db) psum group per window,
            # placed at least one window before its scores consumer.
            q_sched = {0: (0, 1), 1: (0, 2), 2: (0, 3), 3: (1, 0),
                       4: (1, 1), 5: (1, 2), 6: (1, 3), 7: (2, 0),
                       8: (2, 1), 9: (2, 2), 10: (2, 3), 11: (3, 0),
                       12: (3, 1), 13: (3, 2), 14: (3, 3)}
            AVLAG = 6            # last window: inline A.V at this lag

            for w, (nn, hp) in enumerate(windows):
                h0, h1 = 2 * hp, 2 * hp + 1
                s0 = nn * 512
                # Stage xq one window ahead of its first Q-projection
                # window (chunk c's groups run in w4c-2..w4c+1).
                if hp == 1 and nn <= NQC - 2:
                    xq_of[nn + 1] = xsp.tile(
                        [128, EB, 512], XD, tag="xs", name=f"xq{nn+1}"
                    )
                    dma_xchunk(xq_of[nn + 1], xqR, nn + 1)
                if w < len(windows) - 1:
                    yt0 = ytp.tile([65, 512], F32, tag="ytp", name="yt0")
                    yt1 = ytp.tile([65, 512], F32, tag="ytp", name="yt1")
                else:
                    # last window: its A.V runs inline (lag AVLAG) in this
                    # same window, so it needs accumulators disjoint from
                    # ytp (still busy with window w-1) -> use opp's banks.
                    yt0 = opp.tile([65, 512], F32, tag="op", name="yt0i")
                    yt1 = opp.tile([65, 512], F32, tag="op", name="yt1i")
                if hp == 0:
                    yt_sb_of[nn] = ypool.tile(
                        [128, DB, 512], XD, tag="yt", name=f"yt_sb{nn}"
                    )
                e_tiles = []

                for tb in range(TB):
                    # deferred normalize (window w-2's pair)
                    if tb == 0 and norm_q:
                        normalize(*norm_q.pop(0))

                    # scores^T for both heads: two K=64 row-tiles.
                    st = stp.tile([128, 1024], F32, tag="st", name="st")
                    nc.tensor.matmul(
                        st[:, 0:512],
                        kt_sb[0:64, hp, tb * 128 : (tb + 1) * 128],
                        qt_sb[0:64, hp, s0 : s0 + 512],
                        start=True,
                        stop=True,
                        tile_position=(0, 0),
                    )
                    nc.tensor.matmul(
                        st[:, 512:1024],
                        kt_sb[64:128, hp, tb * 128 : (tb + 1) * 128],
                        qt_sb[64:128, hp, s0 : s0 + 512],
                        start=True,
                        stop=True,
                        tile_position=(64, 0),
                    )
                    e_tb = epool.tile([128, 1024], BF16, tag="e")
                    nc.scalar.activation(e_tb[:], st[:], AF.Exp, scale=0.125)
                    e_tiles.append(e_tb)

                    # ---- PE slack extras ----
                    if w in v_sched and tb in v_sched[w]:
                        vtb = v_sched[w][tb]
                        if vtb % 4 == 0:
                            xv_cur = xv_next
                            if vtb < 12:
                                xv_next = xvp.tile(
                                    [128, EB, 512], XD, tag="xv",
                                    name=f"xv{vtb//4+1}",
                                )
                                dma_xchunk(xv_next, xvR, vtb // 4 + 1)
                        v_block(xv_cur, vtb)
                    if w in q_sched and tb < EB:
                        # Q projection, one psum group per window spread
                        # one MM per iteration.
                        qc, qdb = q_sched[w]
                        proj_group(
                            xq_of[qc], wq_sb, bq_sb, qt_sb, qc, qdb,
                            kbr=range(tb, tb + 1),
                        )
                    osched = op_sched_last if nn == NQC - 1 else op_sched
                    if nn >= 1 and hp in osched and tb in osched[hp]:
                        g = osched[hp][tb]
                        outproj_group(
                            yt_sb_of[nn - 1], (nn - 1) * 512, g // 2, g % 2
                        )

                    # ---- deferred A.V of the previous window ----
                    if prev is not None:
                        pe, ph0, ph1, pyt0, pyt1, pnn = prev
                        nc.tensor.matmul(
                            pyt0[:],
                            v_sb[:, tb, ph0 * (HD + 1) : (ph0 + 1) * (HD + 1)],
                            pe[tb][:, 0:512],
                            start=(tb == 0),
                            stop=(tb == TB - 1),
                        )
                        nc.tensor.matmul(
                            pyt1[:],
                            v_sb[:, tb, ph1 * (HD + 1) : (ph1 + 1) * (HD + 1)],
                            pe[tb][:, 512:1024],
                            start=(tb == 0),
                            stop=(tb == TB - 1),
                        )
                    # last window: inline A.V at a small lag so the
                    # epilogue only drains AVLAG t-blocks.
                    if w == len(windows) - 1 and tb >= AVLAG:
                        t2 = tb - AVLAG
                        nc.tensor.matmul(
                            yt0[:],
                            v_sb[:, t2, h0 * (HD + 1) : (h0 + 1) * (HD + 1)],
                            e_tiles[t2][:, 0:512],
                            start=(t2 == 0),
                            stop=False,
                        )
                        nc.tensor.matmul(
                            yt1[:],
                            v_sb[:, t2, h1 * (HD + 1) : (h1 + 1) * (HD + 1)],
                            e_tiles[t2][:, 512:1024],
                            start=(t2 == 0),
                            stop=False,
                        )

                # evacuate the previous window's accumulators, queue norm
                if prev is not None:
                    pe, ph0, ph1, pyt0, pyt1, pnn = prev
                    ye = [
                        npool.tile([65, 512], F32, tag="ye0", name="ye0"),
                        npool.tile([65, 512], F32, tag="ye1", name="ye1"),
                    ]
                    nc.vector.tensor_copy(ye[0][:], pyt0[:])
                    nc.vector.tensor_copy(ye[1][:], pyt1[:])
                    norm_q.append((ye, (ph0, ph1), yt_sb_of[pnn]))
                prev = (e_tiles, h0, h1, yt0, yt1, nn)

            # ---- epilogue: drain the last window's A.V, then finish ----
            if norm_q:
                normalize(*norm_q.pop(0))
            pe, ph0, ph1, pyt0, pyt1, pnn = prev
            for tb in range(TB - AVLAG, TB):
                nc.tensor.matmul(
                    pyt0[:],
                    v_sb[:, tb, ph0 * (HD + 1) : (ph0 + 1) * (HD + 1)],
                    pe[tb][:, 0:512],
                    start=False,
                    stop=(tb == TB - 1),
                )
                nc.tensor.matmul(
                    pyt1[:],
                    v_sb[:, tb, ph1 * (HD + 1) : (ph1 + 1) * (HD + 1)],
                    pe[tb][:, 512:1024],
                    start=False,
                    stop=(tb == TB - 1),
                )
            ye = [
                npool.tile([65, 512], F32, tag="ye0", name="ye0"),
                npool.tile([65, 512], F32, tag="ye1", name="ye1"),
            ]
            nc.vector.tensor_copy(ye[0][:], pyt0[:])
            nc.vector.tensor_copy(ye[1][:], pyt1[:])
            if norm_q:
                normalize(*norm_q.pop(0))
            normalize(ye, (ph0, ph1), yt_sb_of[pnn])
            for g in range(8):
                outproj_group(yt_sb_of[NQC - 1], (NQC - 1) * 512, g // 2, g % 2)

    nc.compile()
    return nc


_NC_CACHE = {}


def _get_nc(key, builder):
    if key not in _NC_CACHE:
        _NC_CACHE[key] = builder()
    return _NC_CACHE[key]


def kernel(query, key, value, Wq, bq, Wk, bk, Wv, bv, Wp, bp):
    query = np.asarray(query, np.float32)
    key = np.asarray(key, np.float32)
    value = np.asarray(value, np.float32)
    Wq, bq = np.asarray(Wq, np.float32), np.asarray(bq, np.float32)
    Wk, bk = np.asarray(Wk, np.float32), np.asarray(bk, np.float32)
    Wv, bv = np.asarray(Wv, np.float32), np.asarray(bv, np.float32)
    Wp, bp = np.asarray(Wp, np.float32), np.asarray(bp, np.float32)

    n, s, e = query.shape
    t = value.shape[1]
    assert (n, s, t, e) == (N_FULL, S_FULL, T_FULL, E_FULL)

    nc = _get_nc(
        "full",
        lambda: _build_nc(E_FULL, S_FULL, T_FULL, H_FULL // 2),
    )

    DG = (H_FULL // 2) * HD
    bf = ml_dtypes.bfloat16
    in_maps = []
    for c in range(N_CORES):
        b, g = c // 2, c % 2
        gs = slice(g * DG, (g + 1) * DG)
        in_maps.append(
            {
                "xqT": query[b].T.astype(bf),
                "xkT": key[b].T.astype(bf),
                "xvT": value[b].T.astype(bf),
                "wqT": Wq[gs, :].T.astype(bf),
                "wkT": Wk[gs, :].T.astype(bf),
                "wvT": Wv[gs, :].T.astype(bf),
                "wpT": Wp[:, gs].T.astype(bf),
                "bq": np.ascontiguousarray(bq[gs]),
                "bk": np.ascontiguousarray(bk[gs]),
                "bv": np.ascontiguousarray(bv[gs]),
            }
        )

    res = run_bass_kernel_spmd(
        nc, in_maps, list(range(N_CORES)), trace=TRACE, **TRACE_KW
    )
    LAST_RESULT[0] = res

    outp = np.empty((n, s, e), np.float32)
    for b in range(n):
        outp[b] = res.results[2 * b]["out"] + res.results[2 * b + 1]["out"] + bp
    return outp
